# revision 3
# baseline (speedup 1.0000x reference)
"""DetectionLoss Bass/Tile kernel v2.1 for TRN2 (one core = one image; SPMD x8).

Windowed exclusive-mask gather design:
- Host sorts anchors by area (loss is order-invariant); IoU>=0.5 requires
  S_anchor in [S_gt/2, 2*S_gt], so gt j only interacts with a contiguous,
  compile-time column window (union over the 8 images for SPMD, widened to
  the union of its block of 4 gts so gather chunks read only written data).
- Phase A per gt j over its window: clamp geometry in f16 (half-scaled),
  m = -inter/4 + SA/12 (Pool), exclusive mask ex_j = (m <= -SG_j/12);
  claimed anchors get the threshold plane shifted by -30000 so later gts
  skip them (first-wins ~= argmax; rel err ~1e-3).
- Gather interleaved into phase A: per 32-anchor group, once the last
  covering gt block is final: PE-transpose the mask block, matmul against
  host-built block-diagonal payload tables (hi+lo accumulated in PSUM)
  -> xg/yg/lwg/lhg, pos count, label per anchor.
- Focal: -N = sigmoid(x)^2*ln(sigmoid(-x)) via 3 activations/class; sums
  accumulated on PE (ones-matmul column sums into PSUM banks).
Output [1,4]: [npos, sl1_sum(*2beta), -nsum, -corr]; host finishes.
"""
import dataclasses
import numpy as np

import concourse.bass as bass
import concourse.mybir as mybir
from concourse import tile

AL = mybir.AluOpType
AF = mybir.ActivationFunctionType
f32 = mybir.dt.float32
f16 = mybir.dt.float16
bf16 = mybir.dt.bfloat16
i16 = mybir.dt.int16

P = 128
G = 32
C = 8
EPS = 1e-7
BETA = 1.0 / 9.0
NB = 4            # gts per window block / gather chunk
NBLK = G // NB    # 8 blocks
MROW = 32         # anchors per transpose chunk (128 // NB)
GW = 32           # anchor cols per gather group (= MROW)
MAXW = 800        # phase-A subrange cap (SBUF temp width)

RF_GX1, RF_GY1, RF_GX2, RF_GY2, RF_NAB3 = 0, 1, 2, 3, 4
NF = 5
NP = 6            # payloads: xg yg lwg lhg cnt lab


def patch_tile_drain(maxw: int = 1):
    import concourse.tile as tile_mod
    from concourse.vector_clock import ScopedClock

    def _drain_and_barrier(self, tick_clock, wait_clock):
        drain_inst = self.nc.sync.drain()
        wait_clock.add_sem_waits(
            drain_inst.ins, ScopedClock({None: tick_clock.global_clock})
        )
        si = drain_inst.ins.sync_info
        waits = list(si.on_wait)
        if len(waits) > maxw:
            si.on_wait = waits[:maxw]
            rest = waits[maxw:]
            for i in range(0, len(rest), maxw):
                nop = self.nc.sync.nop(nofuse=True, hint="drain_split")
                nop.ins.sync_info = mybir.SyncInfo(
                    on_wait=rest[i:i + maxw], on_update=[]
                )
        self.nc.all_engine_barrier()
        assert self.sems is not None
        popped = self.nc._tile_sem_poison_stack.pop()
        assert popped is self._sem_poison
        self.nc.clear_and_free_semaphores(list(self.sems.allocated().values()))
        self.nc.all_engine_barrier()

    tile_mod.TileContext._drain_and_barrier = _drain_and_barrier


def split_sync_waits(nc, maxw: int = 1):
    ctr = [0]

    def mknop(engine, waits):
        ctr[0] += 1
        nop = mybir.InstNoOp(name=f"I-wsplit-{ctr[0]}", ins=[], outs=[])
        nop.engine = engine
        nop.sync_info = mybir.SyncInfo(on_wait=waits, on_update=[])
        return nop

    for blk in nc.bb_map.values():
        bb = blk.bb
        il = bb.instructions
        i = 0
        while i < len(il):
            inst = il[i]
            si = inst.sync_info
            mw = 1 if isinstance(inst, mybir.InstTensorScalarPtr) else maxw
            if si is not None and len(si.on_wait) > mw:
                waits = list(si.on_wait)
                si.on_wait = waits[:mw]
                rest = waits[mw:]
                for k in range(0, len(rest), 1):
                    il.insert(i, mknop(inst.engine, rest[k:k + 1]))
                    i += 1
            i += 1


def build(A, win, groups, debug=False):
    """win: G (c0, c1) block-widened column windows.
    groups: list of (g, [blocks]) per 32-col anchor group."""
    assert A % P == 0
    COLS = A // P
    NG = (COLS + GW - 1) // GW          # 32-col gather groups
    WGP = NG * GW                        # grid/gather pitch
    nc = bass.Bass()
    anch = nc.declare_dram_parameter("anch", [4, A], f32, isOutput=False)
    clsp = nc.declare_dram_parameter("clsp", [C, A], f16, isOutput=False)
    regp = nc.declare_dram_parameter("regp", [4, A], f16, isOutput=False)
    gtf = nc.declare_dram_parameter("gtf", [1, NF * G], f32, isOutput=False)
    ttbs = nc.declare_dram_parameter("ttbs", [P, 2 * NBLK * MROW * NP], f32,
                                     isOutput=False)
    out = nc.declare_dram_parameter("out", [1, 4], f32, isOutput=True)
    if debug:
        dbg = nc.declare_dram_parameter("dbg", [P, 4 * COLS], f32,
                                        isOutput=True)

    def plane(t, c):
        return t[c].rearrange("(p w) -> p w", p=P)

    CH = [(0, 512), (512, 1024), (1024, COLS)]

    ready = {}
    for g, bs in groups:
        if not bs:
            continue
        j_last = max(NB * b + NB - 1 for b in bs)
        ready.setdefault(j_last, []).append((g, bs))

    with tile.TileContext(nc) as tc:
        from contextlib import ExitStack
        with ExitStack() as ctx:
            const = ctx.enter_context(tc.tile_pool(name="const", bufs=1))
            persist = ctx.enter_context(tc.tile_pool(name="persist", bufs=1))

            # ---------- constants ----------
            irow = const.tile([P, P], f32, name="irow")
            nc.gpsimd.iota(irow[:], pattern=[[1, P]], base=0,
                           channel_multiplier=0,
                           allow_small_or_imprecise_dtypes=True)
            icol = const.tile([P, 1], f32, name="icol")
            nc.gpsimd.iota(icol[:], pattern=[[0, 1]], base=0,
                           channel_multiplier=1,
                           allow_small_or_imprecise_dtypes=True)
            identB = const.tile([P, P], bf16, name="identB")
            nc.vector.tensor_scalar(out=identB[:], in0=irow[:],
                                    scalar1=icol[:], scalar2=None,
                                    op0=AL.is_equal)
            ones = const.tile([P, 1], f32, name="ones")
            nc.gpsimd.memset(ones[:], 1.0)
            onesH = const.tile([P, 1], f16, name="onesH")
            nc.gpsimd.memset(onesH[:], 1.0)

            gtfb = const.tile([P, NF * G], f32, name="gtfb")
            gsrc = gtf[:]
            nc.sync.dma_start(
                gtfb[:], dataclasses.replace(gsrc, ap=[[0, P]] + gsrc.ap[1:]))

            def gf(r, j):
                return gtfb[:, r * G + j:r * G + j + 1]

            TBW = MROW * NP
            tt32 = const.tile([P, 2 * NBLK * TBW], f32, name="tt32")
            nc.sync.dma_start(tt32[:], ttbs[:])
            ttb = const.tile([P, 2 * NBLK * TBW], bf16, name="ttb")
            nc.vector.tensor_copy(ttb[:], tt32[:])

            def ttb_hl(b, lo):
                o = (2 * b + lo) * TBW
                return ttb[:, o:o + TBW]

            # ---------- gather output (lives through reg/corr) ----------
            gath_stack = ExitStack()
            gathp = gath_stack.enter_context(
                tc.tile_pool(name="gathp", bufs=1))
            gath = gathp.tile([P, NP, WGP], f16, name="gath")
            gf_flat = gath[:].rearrange("p a b -> p (a b)")
            nc.gpsimd.memset(gf_flat[:, 0:NP * WGP], 0.0)

            # ---------- mask grid ----------
            oh_stack = ExitStack()
            ohp = oh_stack.enter_context(tc.tile_pool(name="ohgrid", bufs=1))
            oh = ohp.tile([P, G, WGP], bf16, name="oh")
            if WGP > COLS:
                flat = oh[:].rearrange("p a b -> p (a b)")
                tail = dataclasses.replace(
                    flat, offset=flat.offset + COLS,
                    ap=[flat.ap[0], [WGP, G], [1, WGP - COLS]])
                nc.gpsimd.memset(tail, 0.0)

            # ---------- anchors + prep ----------
            axh_stack = ExitStack()
            axp = axh_stack.enter_context(tc.tile_pool(name="axp", bufs=1))
            pA_stack = ExitStack()
            pA = pA_stack.enter_context(tc.tile_pool(name="pA", bufs=1))
            ax = [pA.tile([P, COLS], f32, name=f"ax{i}") for i in range(4)]
            PC0 = 384
            for u0, u1 in ((0, PC0), (PC0, COLS)):
                for i in range(4):
                    nc.sync.dma_start(ax[i][:, u0:u1],
                                      plane(anch, i)[:, u0:u1])
            axh = [axp.tile([P, COLS], f16, name=f"axh{i}") for i in range(4)]
            wah = pA.tile([P, COLS], f16, name="wah")
            hah = pA.tile([P, COLS], f16, name="hah")
            sa3 = pA.tile([P, COLS], f16, name="sa3")
            negq = persist.tile([P, COLS], f16, name="negq")
            for u0, u1 in ((0, PC0), (PC0, COLS)):
                sl = slice(u0, u1)
                for i in range(4):
                    nc.scalar.activation(axh[i][:, sl], ax[i][:, sl],
                                         AF.Identity, scale=0.5)
                nc.vector.tensor_tensor(out=wah[:, sl], in0=axh[2][:, sl],
                                        in1=axh[0][:, sl], op=AL.subtract)
                nc.vector.tensor_tensor(out=hah[:, sl], in0=axh[3][:, sl],
                                        in1=axh[1][:, sl], op=AL.subtract)
                nc.vector.tensor_tensor(out=sa3[:, sl], in0=wah[:, sl],
                                        in1=hah[:, sl], op=AL.mult)
                nc.scalar.activation(negq[:, sl], sa3[:, sl], AF.Identity,
                                     scale=-1.0 / 3.0)
            ca = persist.tile([P, COLS], f16, name="ca")
            ya = persist.tile([P, COLS], f16, name="ya")
            nc.vector.tensor_tensor(out=ca[:], in0=axh[0][:], in1=axh[2][:],
                                    op=AL.add)
            nc.vector.tensor_tensor(out=ya[:], in0=axh[1][:], in1=axh[3][:],
                                    op=AL.add)
            wae = pA.tile([P, COLS], f32, name="wae")
            hae = pA.tile([P, COLS], f32, name="hae")
            nc.vector.tensor_scalar(out=wae[:], in0=wah[:], scalar1=2.0,
                                    scalar2=EPS, op0=AL.mult, op1=AL.add)
            nc.vector.tensor_scalar(out=hae[:], in0=hah[:], scalar1=2.0,
                                    scalar2=EPS, op0=AL.mult, op1=AL.add)
            iwa = persist.tile([P, COLS], f16, name="iwa")
            iha = persist.tile([P, COLS], f16, name="iha")
            with nc.allow_low_precision(reason="f16 reg targets"):
                nc.vector.reciprocal(iwa[:], wae[:])
                nc.vector.reciprocal(iha[:], hae[:])
            La = persist.tile([P, COLS], f16, name="La")
            Ha = persist.tile([P, COLS], f16, name="Ha")
            nc.scalar.activation(La[:], wae[:], AF.Ln)
            nc.scalar.activation(Ha[:], hae[:], AF.Ln)
            pA_stack.close()

            # ---------- psum accumulators (phase A) ----------
            acc_stack = ExitStack()
            accp = acc_stack.enter_context(
                tc.tile_pool(name="accp", bufs=1, space="PSUM"))
            nsumP = accp.tile([1, 512], f32, name="nsumP")

            gp_stack = ExitStack()
            psum_t = gp_stack.enter_context(
                tc.tile_pool(name="psum_t", bufs=2, space="PSUM"))
            psum_g = gp_stack.enter_context(
                tc.tile_pool(name="psum_g", bufs=3, space="PSUM"))
            ohtp = gp_stack.enter_context(tc.tile_pool(name="ohtp", bufs=2))

            ohbase = oh[:].rearrange("p a b -> p (a b)")

            def emit_group(g, bs):
                nch = len(bs)
                c0g = g * GW
                pt = psum_t.tile([P, 128 * nch], bf16, name="pt")
                for q, b in enumerate(bs):
                    src = dataclasses.replace(
                        ohbase,
                        offset=ohbase.offset + (NB * b) * WGP + c0g,
                        ap=[ohbase.ap[0], [WGP, NB], [1, MROW]])
                    nc.tensor.transpose(
                        pt[:, 128 * q:128 * q + 128], src, identB[:])
                ohT = ohtp.tile([P, 128 * nch], bf16, name="ohT")
                nc.scalar.copy(ohT[:], pt[:])
                gp = psum_g.tile([P, MROW * NP], f32, name="gp")
                for q, b in enumerate(bs):
                    l = ohT[:, 128 * q:128 * q + 128]
                    nc.tensor.matmul(out=gp[:], lhsT=l, rhs=ttb_hl(b, 0),
                                     start=(q == 0), stop=False)
                    nc.tensor.matmul(out=gp[:], lhsT=l, rhs=ttb_hl(b, 1),
                                     start=False, stop=(q == nch - 1))
                dst = dataclasses.replace(
                    gf_flat, offset=gf_flat.offset + c0g,
                    ap=[gf_flat.ap[0], [1, MROW], [WGP, NP]])
                nc.scalar.copy(dst, gp[:])

            # ---------- phase A ----------
            fns_stack = ExitStack()
            fs1 = fns_stack.enter_context(tc.tile_pool(name="fns", bufs=1))
            fxc = [fs1.tile([P, COLS], f16, name=f"fxc{i}") for i in range(2)]
            fsg = fs1.tile([P, COLS], f16, name="fsg")
            fsp = fsg
            fs2 = fs1.tile([P, COLS], f16, name="fs2")
            fNo = [fs1.tile([P, COLS], f16, name=f"No{i}") for i in range(2)]

            def emit_focal_class(c):
                # No = -N(x) = (1-sgn)^2 * ln(sgn), sgn = sigmoid(-x)
                i2 = c % 2
                xc = fxc[i2]
                nc.sync.dma_start(xc[:], plane(clsp, c))
                nc.scalar.activation(fsg[:], xc[:], AF.Sigmoid, scale=-1.0)
                nc.scalar.activation(fs2[:], fsg[:], AF.Square,
                                     scale=-1.0, bias=1.0)
                nc.scalar.activation(fsp[:], fsg[:], AF.Ln)
                nc.gpsimd.tensor_tensor(out=fNo[i2][:], in0=fs2[:],
                                        in1=fsp[:], op=AL.mult)
                for k, (u0, u1) in enumerate(CH):
                    nc.tensor.matmul(
                        out=nsumP[:, 0:u1 - u0], lhsT=onesH[:],
                        rhs=fNo[i2][:, u0:u1],
                        start=(c == 0 and k == 0),
                        stop=(c == C - 1 and k == len(CH) - 1))

            with tc.tile_pool(name="jtmp", bufs=1) as jt:
                ltx2 = [jt.tile([P, MAXW], f16, name=f"ltx{i}") for i in range(2)]
                ux2 = [jt.tile([P, MAXW], f16, name=f"ux{i}") for i in range(2)]
                lty2 = [jt.tile([P, MAXW], f16, name=f"lty{i}") for i in range(2)]
                vy2 = [jt.tile([P, MAXW], f16, name=f"vy{i}") for i in range(2)]
                wxp2 = ltx2
                wyn2 = lty2
                int3 = [jt.tile([P, MAXW], f16, name=f"int{i}") for i in range(3)]
                m3 = [jt.tile([P, MAXW], f16, name=f"m{i}") for i in range(3)]
                pend = None
                nsub = [0]

                def flush_tail():
                    nonlocal pend
                    if pend is None:
                        return
                    j, c0, c1, mj, k3 = pend
                    pend = None
                    w = c1 - c0
                    eb = eb3[k3 % 2]
                    nc.vector.tensor_scalar(out=oh[:, j, c0:c1],
                                            in0=mj[:, 0:w],
                                            scalar1=gf(RF_NAB3, j),
                                            scalar2=None, op0=AL.is_le)
                    nc.vector.tensor_scalar(out=eb[:, 0:w], in0=mj[:, 0:w],
                                            scalar1=gf(RF_NAB3, j),
                                            scalar2=30000.0,
                                            op0=AL.is_le, op1=AL.mult)
                    nc.vector.tensor_tensor(out=negq[:, c0:c1],
                                            in0=negq[:, c0:c1],
                                            in1=eb[:, 0:w], op=AL.subtract)

                def emit_sub(j, c0, c1):
                    sl = slice(c0, c1)
                    w = c1 - c0
                    k2 = nsub[0] % 2
                    k3 = nsub[0] % 3
                    ltx = ltx2[k2]; ux = ux2[k2]
                    lty = lty2[k2]; vy = vy2[k2]
                    wxp = wxp2[k2]; wyn = wyn2[k2]
                    itg = int3[k3]; mj = m3[k3]
                    v = nc.vector
                    v.tensor_scalar(out=ltx[:, 0:w], in0=axh[0][:, sl],
                                    scalar1=gf(RF_GX1, j), scalar2=gf(RF_GX2, j),
                                    op0=AL.max, op1=AL.min)
                    v.tensor_scalar(out=ux[:, 0:w], in0=axh[2][:, sl],
                                    scalar1=gf(RF_GX2, j), scalar2=gf(RF_GX1, j),
                                    op0=AL.min, op1=AL.max)
                    v.tensor_scalar(out=lty[:, 0:w], in0=axh[1][:, sl],
                                    scalar1=gf(RF_GY1, j), scalar2=gf(RF_GY2, j),
                                    op0=AL.max, op1=AL.min)
                    v.tensor_scalar(out=vy[:, 0:w], in0=axh[3][:, sl],
                                    scalar1=gf(RF_GY2, j), scalar2=gf(RF_GY1, j),
                                    op0=AL.min, op1=AL.max)
                    v.tensor_tensor(out=wxp[:, 0:w], in0=ux[:, 0:w],
                                    in1=ltx[:, 0:w], op=AL.subtract)
                    v.tensor_tensor(out=wyn[:, 0:w], in0=lty[:, 0:w],
                                    in1=vy[:, 0:w], op=AL.subtract)
                    v.tensor_tensor(out=itg[:, 0:w], in0=wxp[:, 0:w],
                                    in1=wyn[:, 0:w], op=AL.mult)
                    flush_tail()
                    nc.gpsimd.tensor_tensor(out=mj[:, 0:w], in0=itg[:, 0:w],
                                            in1=negq[:, sl], op=AL.subtract)
                    return mj, k3

                for j in range(G):
                    c0, c1 = win[j]
                    s0 = c0
                    while s0 < c1:
                        s1 = min(s0 + MAXW, c1)
                        mj, k3 = emit_sub(j, s0, s1)
                        pend = (j, s0, s1, mj, k3)
                        nsub[0] += 1
                        s0 = s1
                    if j % NB == NB - 1:
                        emit_focal_class(j // NB)
                    if j in ready:
                        flush_tail()
                        for g, bs in ready[j]:
                            emit_group(g, bs)
                flush_tail()
            fns_stack.close()

            # read out nsum; close gather pools
            nsum1 = persist.tile([1, 1], f32, name="nsum1")
            nscr = persist.tile([1, 512], f32, name="nscr")
            nc.scalar.activation(nscr[:], nsumP[:], AF.Identity,
                                 accum_out=nsum1[:])
            gp_stack.close()
            acc_stack.close()
            axh_stack.close()
            oh_stack.close()

            def gpl(idx):
                return gath[:, idx, 0:COLS]

            pos = gpl(4)
            labf = gpl(5)

            nposA = persist.tile([P, 1], f32, name="nposA")
            with tc.tile_pool(name="pscrp", bufs=1) as pscrp:
                pscr = pscrp.tile([P, COLS], f16, name="pscr")
                nc.scalar.activation(pscr[:], pos, AF.Identity,
                                     accum_out=nposA[:])

            if debug:
                dview = dbg[:].rearrange("p (n w) -> p n w", n=4)
                nc.scalar.copy(dview[:, 0, :], pos)
                nc.scalar.copy(dview[:, 1, :], labf)
                nc.scalar.copy(dview[:, 2, :], gpl(0))
                nc.scalar.copy(dview[:, 3, :], gpl(2))

            acc2_stack = ExitStack()
            accp2 = acc2_stack.enter_context(
                tc.tile_pool(name="accp2", bufs=1, space="PSUM"))
            sl1P = accp2.tile([1, 512], f32, name="sl1P")
            roP = accp2.tile([1, 512], f32, name="roP")

            # ---------- reg smooth-L1 ----------
            with ExitStack() as rctx:
                rp_p = rctx.enter_context(tc.tile_pool(name="rp", bufs=2))
                rs = rctx.enter_context(tc.tile_pool(name="rs", bufs=1))
                dv = [rs.tile([P, COLS], f16, name=f"dv{i}") for i in range(2)]
                rt = [rs.tile([P, COLS], f16, name=f"rt{i}") for i in range(2)]
                ef = [rs.tile([P, COLS], f16, name=f"ef{i}") for i in range(2)]
                qf = [rs.tile([P, COLS], f16, name=f"qf{i}") for i in range(2)]
                qm = [rs.tile([P, COLS], f16, name=f"qm{i}") for i in range(2)]
                cm = [rs.tile([P, COLS], f16, name=f"cm{i}") for i in range(2)]
                q2 = [rs.tile([P, COLS], f16, name=f"q2{i}") for i in range(2)]
                t2 = [rs.tile([P, COLS], f16, name=f"t2{i}") for i in range(2)]
                so = [rs.tile([P, COLS], f16, name=f"so{i}") for i in range(2)]
                for k, (gi, ctr, inv, lg) in enumerate((
                        (0, ca, iwa, None), (1, ya, iha, None),
                        (2, None, None, La), (3, None, None, Ha))):
                    i2 = k % 2
                    g = gpl(gi)
                    if lg is None:
                        nc.vector.tensor_tensor(out=dv[i2][:], in0=g,
                                                in1=ctr[:], op=AL.subtract)
                        nc.vector.tensor_tensor(out=rt[i2][:], in0=dv[i2][:],
                                                in1=inv[:], op=AL.mult)
                    else:
                        nc.vector.tensor_tensor(out=rt[i2][:], in0=g,
                                                in1=lg[:], op=AL.subtract)
                    rp = rp_p.tile([P, COLS], f16, name="rp")
                    nc.sync.dma_start(rp[:], plane(regp, k))
                    nc.vector.tensor_tensor(out=ef[i2][:], in0=rp[:],
                                            in1=rt[i2][:], op=AL.subtract)
                    nc.scalar.activation(qf[i2][:], ef[i2][:], AF.Abs)
                    nc.gpsimd.tensor_tensor(out=qm[i2][:], in0=qf[i2][:],
                                            in1=pos, op=AL.mult)
                    nc.vector.tensor_scalar(out=cm[i2][:], in0=qm[i2][:],
                                            scalar1=BETA, scalar2=None,
                                            op0=AL.min)
                    nc.vector.tensor_scalar(out=q2[i2][:], in0=qm[i2][:],
                                            scalar1=2.0, scalar2=None,
                                            op0=AL.mult)
                    nc.vector.tensor_tensor(out=t2[i2][:], in0=q2[i2][:],
                                            in1=cm[i2][:], op=AL.subtract)
                    nc.vector.tensor_tensor(out=so[i2][:], in0=cm[i2][:],
                                            in1=t2[i2][:], op=AL.mult)
                    for kk, (u0, u1) in enumerate(CH):
                        nc.tensor.matmul(
                            out=sl1P[:, 0:u1 - u0], lhsT=onesH[:],
                            rhs=so[i2][:, u0:u1],
                            start=(k == 0 and kk == 0),
                            stop=(k == 3 and kk == len(CH) - 1))

            # ---------- focal corr ----------
            with ExitStack() as fctx:
                fs = fctx.enter_context(tc.tile_pool(name="fsc", bufs=1))
                x_lab = fs.tile([P, COLS], f16, name="x_lab")
                nc.scalar.activation(x_lab[:], x_lab[:], AF.MemsetZero,
                                     zero_input=True) if False else \
                    nc.gpsimd.memset(x_lab[:], 0.0)
                mk = [fs.tile([P, COLS], i16, name=f"mk{i}") for i in range(2)]
                cxc = [fs.tile([P, COLS], f16, name=f"cxc{i}") for i in range(2)]
                for c in range(C):
                    i2 = c % 2
                    nc.sync.dma_start(cxc[i2][:], plane(clsp, c))
                    nc.vector.tensor_scalar(out=mk[i2][:], in0=labf,
                                            scalar1=float(c), scalar2=None,
                                            op0=AL.is_equal)
                    nc.vector.copy_predicated(out=x_lab[:], mask=mk[i2][:],
                                              data=cxc[i2][:])
                # R' = -R = (-P)/3 - (-N); -P = sgn^2*ln(sg); -N = sg^2*ln(sgn)
                sgn = fs.tile([P, COLS], f16, name="sgn")
                sgl = fs.tile([P, COLS], f16, name="sgl")
                lgn = fs.tile([P, COLS], f16, name="lgn")
                lgl = fs.tile([P, COLS], f16, name="lgl")
                a2 = fs.tile([P, COLS], f16, name="a2")
                Pl = fs.tile([P, COLS], f16, name="Pl")
                b2 = fs.tile([P, COLS], f16, name="b2")
                Nl = fs.tile([P, COLS], f16, name="Nl")
                Rl = fs.tile([P, COLS], f16, name="Rl")
                Ro = fs.tile([P, COLS], f16, name="Ro")
                nc.scalar.activation(sgn[:], x_lab[:], AF.Sigmoid, scale=-1.0)
                nc.scalar.activation(sgl[:], x_lab[:], AF.Sigmoid)
                nc.scalar.activation(lgn[:], sgn[:], AF.Ln)
                nc.scalar.activation(lgl[:], sgl[:], AF.Ln)
                nc.vector.tensor_tensor(out=a2[:], in0=sgn[:], in1=sgn[:],
                                        op=AL.mult)
                nc.vector.tensor_tensor(out=Pl[:], in0=a2[:], in1=lgl[:],
                                        op=AL.mult)
                nc.vector.tensor_tensor(out=b2[:], in0=sgl[:], in1=sgl[:],
                                        op=AL.mult)
                nc.vector.tensor_tensor(out=Nl[:], in0=b2[:], in1=lgn[:],
                                        op=AL.mult)
                nc.vector.scalar_tensor_tensor(out=Rl[:], in0=Pl[:],
                                               scalar=1.0 / 3.0, in1=Nl[:],
                                               op0=AL.mult, op1=AL.subtract)
                nc.vector.tensor_tensor(out=Ro[:], in0=Rl[:], in1=pos,
                                        op=AL.mult)
                for kk, (u0, u1) in enumerate(CH):
                    nc.tensor.matmul(
                        out=roP[:, 0:u1 - u0], lhsT=onesH[:],
                        rhs=Ro[:, u0:u1],
                        start=(kk == 0), stop=(kk == len(CH) - 1))

            # ---------- final reduce ----------
            sl11 = persist.tile([1, 1], f32, name="sl11")
            ro1 = persist.tile([1, 1], f32, name="ro1")
            scr1 = persist.tile([1, 512], f32, name="scr1")
            nc.scalar.activation(scr1[:], sl1P[:], AF.Identity,
                                 accum_out=sl11[:])
            scr2 = persist.tile([1, 512], f32, name="scr2")
            nc.scalar.activation(scr2[:], roP[:], AF.Identity,
                                 accum_out=ro1[:])
            acc2_stack.close()
            gath_stack.close()
            with tc.tile_pool(name="psum_f", bufs=1, space="PSUM") as pf:
                fps = pf.tile([1, 1], f32, name="fps")
                nc.tensor.matmul(out=fps[:], lhsT=ones[:], rhs=nposA[:],
                                 start=True, stop=True)
                osb = persist.tile([1, 4], f32, name="osb")
                nc.scalar.copy(osb[:, 0:1], fps[:])
                nc.scalar.copy(osb[:, 1:2], sl11[:])
                nc.scalar.copy(osb[:, 2:3], nsum1[:])
                nc.scalar.copy(osb[:, 3:4], ro1[:])
                nc.sync.dma_start(out[:], osb[:])

    return nc


# ---------------- host side ----------------

def prep_host(cls_preds, reg_preds, anchors, gt_boxes, gt_labels):
    B, A, _ = cls_preds.shape
    COLS = A // P
    NG = (COLS + GW - 1) // GW
    f = np.float32
    a = anchors.astype(np.float64)
    SA = (a[:, 2] - a[:, 0]) * (a[:, 3] - a[:, 1])
    order = np.argsort(SA, kind="stable")
    SA_s = SA[order]

    def cm_layout(x):
        return np.ascontiguousarray(x.reshape(COLS, P).T)

    a_s = a[order]
    anch = np.stack([cm_layout(a_s[:, i].astype(f)) for i in range(4)], 0)
    anch = anch.reshape(4, A)

    import ml_dtypes
    bfc = lambda v: f(f(v).astype(ml_dtypes.bfloat16))

    lo_cols = np.zeros((B, G), np.int64)
    hi_cols = np.zeros((B, G), np.int64)
    maps = []
    for b in range(B):
        gb = gt_boxes[b].astype(np.float64)
        lab = gt_labels[b].astype(np.int64)
        SG = (gb[:, 2] - gb[:, 0]) * (gb[:, 3] - gb[:, 1])
        gorder = np.argsort(SG, kind="stable")
        gb, lab, SG = gb[gorder], lab[gorder], SG[gorder]
        lo = np.searchsorted(SA_s, SG / 2.0)
        hi = np.searchsorted(SA_s, 2.0 * SG, side="right")
        lo_cols[b] = lo // P
        hi_cols[b] = np.minimum((hi + P - 1) // P, COLS)

        gx1, gy1, gx2, gy2 = gb[:, 0], gb[:, 1], gb[:, 2], gb[:, 3]
        rows = np.zeros((NF, G), f)
        rows[RF_GX1] = f(gx1 * 0.5)
        rows[RF_GY1] = f(gy1 * 0.5)
        rows[RF_GX2] = f(gx2 * 0.5)
        rows[RF_GY2] = f(gy2 * 0.5)
        rows[RF_NAB3] = f(-(SG / 12.0))
        xg = (gx1 + gx2) / 2.0
        yg = (gy1 + gy2) / 2.0
        lwg = np.log(gx2 - gx1)
        lhg = np.log(gy2 - gy1)
        pay = np.zeros((2, G, NP), f)
        for r, v in ((0, xg), (1, yg), (2, lwg), (3, lhg)):
            h = bfc(v)
            pay[0, :, r] = h
            pay[1, :, r] = bfc(v - h)
        pay[0, :, 4] = 1.0
        pay[0, :, 5] = lab.astype(f)
        pay /= 32768.0
        TBW = MROW * NP
        tt = np.zeros((P, 2 * NBLK * TBW), f)
        for bb_ in range(NBLK):
            for hl in (0, 1):
                t = np.zeros((P, TBW), f)
                for r in range(NB):
                    j = NB * bb_ + r
                    for i in range(MROW):
                        t[MROW * r + i, NP * i:NP * i + NP] = pay[hl, j]
                tt[:, (2 * bb_ + hl) * TBW:(2 * bb_ + hl + 1) * TBW] = t

        clspb = cls_preds[b][order].astype(np.float16)
        regpb = reg_preds[b][order].astype(np.float16)
        clsp = np.stack([cm_layout(clspb[:, i]) for i in range(C)], 0)
        regp = np.stack([cm_layout(regpb[:, i]) for i in range(4)], 0)
        maps.append({"anch": anch, "clsp": clsp.reshape(C, A),
                     "regp": regp.reshape(4, A),
                     "gtf": rows.reshape(1, -1), "ttbs": tt})

    win = []
    spans = []
    for bb_ in range(NBLK):
        js = range(NB * bb_, NB * bb_ + NB)
        c0 = int(min(lo_cols[:, j].min() for j in js))
        c1 = int(max(hi_cols[:, j].max() for j in js))
        c0 = (max(0, c0) // GW) * GW
        c1 = min(COLS, ((max(c1, c0) + GW - 1) // GW) * GW)
        spans.append((c0, c1))
    for j in range(G):
        win.append(spans[j // NB])
    groups = []
    for g in range(NG):
        g0, g1 = g * GW, min((g + 1) * GW, COLS)
        bs = [bb_ for bb_, (c0, c1) in enumerate(spans)
              if c0 < g1 and c1 > g0]
        groups.append((g, bs))
    return maps, win, groups


def finish(partials):
    f = np.float32
    npos = f(0); sl1 = f(0); nsum = f(0); corr = f(0)
    for p in partials:
        p = p.reshape(4)
        npos += f(p[0]); sl1 += f(p[1]); nsum -= f(p[2]); corr -= f(p[3])
    denom = max(float(npos), 1.0)
    if npos > 0:
        cls_loss = f(0.75) * (nsum + corr) / f(denom)
        reg_loss = sl1 / f(2 * BETA) / f(denom)
    else:
        cls_loss = f(0.0); reg_loss = f(0.0)
    return np.float32(cls_loss), np.float32(reg_loss)


# ---------------- self-contained kernel entry ----------------

_CACHE = {}


def _get_fn(nc, n_cores=8):
    import jax
    from jax.sharding import Mesh, PartitionSpec, NamedSharding
    from jax.experimental.shard_map import shard_map
    from concourse.bass2jax import (_bass_exec_p, install_neuronx_cc_hook,
                                    partition_id_tensor)
    install_neuronx_cc_hook()
    in_names, out_names, out_avals, zero_shapes = [], [], [], []
    partition_name = (nc.partition_id_tensor.name
                      if nc.partition_id_tensor else None)
    for alloc in nc.m.functions[0].allocations:
        if not isinstance(alloc, mybir.MemoryLocationSet):
            continue
        name = alloc.memorylocations[0].name
        if alloc.kind == "ExternalInput":
            if name != partition_name:
                in_names.append(name)
        elif alloc.kind == "ExternalOutput":
            out_names.append(name)
            shape = tuple(alloc.tensor_shape)
            dtype = mybir.dt.np(alloc.dtype)
            out_avals.append(jax.core.ShapedArray(shape, dtype))
            zero_shapes.append((shape, dtype))
    n_params = len(in_names)
    n_outs = len(out_avals)
    all_in_names = in_names + out_names + ([partition_name]
                                           if partition_name else [])
    donate = tuple(range(n_params, n_params + n_outs))

    def _body(*args):
        operands = list(args)
        if partition_name is not None:
            operands.append(partition_id_tensor())
        outs = _bass_exec_p.bind(
            *operands, out_avals=tuple(out_avals),
            in_names=tuple(all_in_names), out_names=tuple(out_names),
            lowering_input_output_aliases=(),
            sim_require_finite=True, sim_require_nnan=True, nc=nc)
        return tuple(outs)

    devices = jax.devices()[:n_cores]
    mesh = Mesh(np.asarray(devices), ("core",))
    in_specs = (PartitionSpec("core"),) * (n_params + n_outs)
    out_specs = (PartitionSpec("core"),) * len(out_names)
    fn = jax.jit(shard_map(_body, mesh=mesh, in_specs=in_specs,
                           out_specs=out_specs, check_rep=False),
                 donate_argnums=donate, keep_unused=True)
    sh = NamedSharding(mesh, PartitionSpec("core"))
    return (fn, in_names, out_names, out_avals, zero_shapes, sh, n_cores)


def kernel(cls_preds, reg_preds, anchors, gt_boxes, gt_labels):
    import jax
    cls_preds = np.asarray(cls_preds)
    reg_preds = np.asarray(reg_preds)
    anchors = np.asarray(anchors)
    gt_boxes = np.asarray(gt_boxes)
    gt_labels = np.asarray(gt_labels)
    B, A, _ = cls_preds.shape
    assert (B, A) == (8, 160000), (B, A)
    maps, win, groups = prep_host(cls_preds, reg_preds, anchors, gt_boxes,
                                  gt_labels)
    key = ("fn", tuple(win), tuple((g, tuple(bs)) for g, bs in groups))
    if key not in _CACHE:
        patch_tile_drain(1)
        nc = build(A, win, groups)
        split_sync_waits(nc)
        _CACHE.clear()
        _CACHE[key] = _get_fn(nc)
    fn, in_names, out_names, out_avals, zero_shapes, sh, n_cores = _CACHE[key]
    concat_in = [jax.device_put(
        np.concatenate([np.asarray(maps[c][nm]) for c in range(n_cores)],
                       axis=0), sh) for nm in in_names]
    zeros = [jax.device_put(
        np.zeros((n_cores * s[0], *s[1:]), d), sh) for s, d in zero_shapes]
    out_arrs = fn(*concat_in, *zeros)
    res = np.asarray(out_arrs[out_names.index("out")]).reshape(n_cores, 1, 4)
    partials = [res[c] for c in range(n_cores)]
    cls_loss, reg_loss = finish(partials)
    return cls_loss, reg_loss


# revision 4
# speedup vs baseline: 1.0187x; 1.0187x over previous
"""DetectionLoss Bass/Tile kernel v2.1 for TRN2 (one core = one image; SPMD x8).

Windowed exclusive-mask gather design:
- Host sorts anchors by area (loss is order-invariant); IoU>=0.5 requires
  S_anchor in [S_gt/2, 2*S_gt], so gt j only interacts with a contiguous,
  compile-time column window (union over the 8 images for SPMD, widened to
  the union of its block of 4 gts so gather chunks read only written data).
- Phase A per gt j over its window: clamp geometry in f16 (half-scaled),
  m = -inter/4 + SA/12 (Pool), exclusive mask ex_j = (m <= -SG_j/12);
  claimed anchors get the threshold plane shifted by -30000 so later gts
  skip them (first-wins ~= argmax; rel err ~1e-3).
- Gather interleaved into phase A: per 32-anchor group, once the last
  covering gt block is final: PE-transpose the mask block, matmul against
  host-built block-diagonal payload tables (hi+lo accumulated in PSUM)
  -> xg/yg/lwg/lhg, pos count, label per anchor.
- Focal: -N = sigmoid(x)^2*ln(sigmoid(-x)) via 3 activations/class; sums
  accumulated on PE (ones-matmul column sums into PSUM banks).
Output [1,4]: [npos, sl1_sum(*2beta), -nsum, -corr]; host finishes.
"""
import dataclasses
import numpy as np

import concourse.bass as bass
import concourse.mybir as mybir
from concourse import tile

AL = mybir.AluOpType
AF = mybir.ActivationFunctionType
f32 = mybir.dt.float32
f16 = mybir.dt.float16
bf16 = mybir.dt.bfloat16
i16 = mybir.dt.int16

P = 128
G = 32
C = 8
EPS = 1e-7
BETA = 1.0 / 9.0
NB = 4            # gts per window block / gather chunk
NBLK = G // NB    # 8 blocks
MROW = 32         # anchors per transpose chunk (128 // NB)
GW = 32           # anchor cols per gather group (= MROW)
MAXW = 800        # phase-A subrange cap (SBUF temp width)

RF_GX1, RF_GY1, RF_GX2, RF_GY2, RF_NAB3 = 0, 1, 2, 3, 4
NF = 5
NP = 6            # payloads: xg yg lwg lhg cnt lab


def patch_tile_drain(maxw: int = 1):
    import concourse.tile as tile_mod
    from concourse.vector_clock import ScopedClock

    def _drain_and_barrier(self, tick_clock, wait_clock):
        drain_inst = self.nc.sync.drain()
        wait_clock.add_sem_waits(
            drain_inst.ins, ScopedClock({None: tick_clock.global_clock})
        )
        si = drain_inst.ins.sync_info
        waits = list(si.on_wait)
        if len(waits) > maxw:
            si.on_wait = waits[:maxw]
            rest = waits[maxw:]
            for i in range(0, len(rest), maxw):
                nop = self.nc.sync.nop(nofuse=True, hint="drain_split")
                nop.ins.sync_info = mybir.SyncInfo(
                    on_wait=rest[i:i + maxw], on_update=[]
                )
        self.nc.all_engine_barrier()
        assert self.sems is not None
        popped = self.nc._tile_sem_poison_stack.pop()
        assert popped is self._sem_poison
        self.nc.clear_and_free_semaphores(list(self.sems.allocated().values()))
        self.nc.all_engine_barrier()

    tile_mod.TileContext._drain_and_barrier = _drain_and_barrier


def split_sync_waits(nc, maxw: int = 1):
    ctr = [0]

    def mknop(engine, waits):
        ctr[0] += 1
        nop = mybir.InstNoOp(name=f"I-wsplit-{ctr[0]}", ins=[], outs=[])
        nop.engine = engine
        nop.sync_info = mybir.SyncInfo(on_wait=waits, on_update=[])
        return nop

    for blk in nc.bb_map.values():
        bb = blk.bb
        il = bb.instructions
        i = 0
        while i < len(il):
            inst = il[i]
            si = inst.sync_info
            mw = 1 if isinstance(inst, mybir.InstTensorScalarPtr) else maxw
            if si is not None and len(si.on_wait) > mw:
                waits = list(si.on_wait)
                si.on_wait = waits[:mw]
                rest = waits[mw:]
                for k in range(0, len(rest), 1):
                    il.insert(i, mknop(inst.engine, rest[k:k + 1]))
                    i += 1
            i += 1


def build(A, win, groups, debug=False):
    """win: G (c0, c1) block-widened column windows.
    groups: list of (g, [blocks]) per 32-col anchor group."""
    assert A % P == 0
    COLS = A // P
    NG = (COLS + GW - 1) // GW          # 32-col gather groups
    WGP = NG * GW                        # grid/gather pitch
    nc = bass.Bass()
    anch = nc.declare_dram_parameter("anch", [4, A], f32, isOutput=False)
    clsp = nc.declare_dram_parameter("clsp", [C, A], f16, isOutput=False)
    regp = nc.declare_dram_parameter("regp", [4, A], f16, isOutput=False)
    gtf = nc.declare_dram_parameter("gtf", [1, NF * G], f32, isOutput=False)
    ttbs = nc.declare_dram_parameter("ttbs", [P, 2 * NBLK * MROW * NP], f32,
                                     isOutput=False)
    out = nc.declare_dram_parameter("out", [1, 4], f32, isOutput=True)
    if debug:
        dbg = nc.declare_dram_parameter("dbg", [P, 4 * COLS], f32,
                                        isOutput=True)

    def plane(t, c):
        return t[c].rearrange("(p w) -> p w", p=P)

    CH = [(0, 512), (512, 1024), (1024, COLS)]

    ready = {}
    for g, bs in groups:
        if not bs:
            continue
        j_last = max(NB * b + NB - 1 for b in bs)
        ready.setdefault(j_last, []).append((g, bs))

    with tile.TileContext(nc) as tc:
        from contextlib import ExitStack
        with ExitStack() as ctx:
            const = ctx.enter_context(tc.tile_pool(name="const", bufs=1))
            persist = ctx.enter_context(tc.tile_pool(name="persist", bufs=1))

            # ---------- constants ----------
            irow = const.tile([P, P], f32, name="irow")
            nc.gpsimd.iota(irow[:], pattern=[[1, P]], base=0,
                           channel_multiplier=0,
                           allow_small_or_imprecise_dtypes=True)
            icol = const.tile([P, 1], f32, name="icol")
            nc.gpsimd.iota(icol[:], pattern=[[0, 1]], base=0,
                           channel_multiplier=1,
                           allow_small_or_imprecise_dtypes=True)
            identB = const.tile([P, P], bf16, name="identB")
            nc.vector.tensor_scalar(out=identB[:], in0=irow[:],
                                    scalar1=icol[:], scalar2=None,
                                    op0=AL.is_equal)
            ones = const.tile([P, 1], f32, name="ones")
            nc.gpsimd.memset(ones[:], 1.0)
            onesH = const.tile([P, 1], f16, name="onesH")
            nc.gpsimd.memset(onesH[:], 1.0)

            gtfb = const.tile([P, NF * G], f32, name="gtfb")
            gsrc = gtf[:]
            nc.sync.dma_start(
                gtfb[:], dataclasses.replace(gsrc, ap=[[0, P]] + gsrc.ap[1:]))

            def gf(r, j):
                return gtfb[:, r * G + j:r * G + j + 1]

            TBW = MROW * NP
            tt32 = const.tile([P, 2 * NBLK * TBW], f32, name="tt32")
            nc.sync.dma_start(tt32[:], ttbs[:])
            ttb = const.tile([P, 2 * NBLK * TBW], bf16, name="ttb")
            nc.vector.tensor_copy(ttb[:], tt32[:])

            def ttb_hl(b, lo):
                o = (2 * b + lo) * TBW
                return ttb[:, o:o + TBW]

            # ---------- gather output (lives through reg/corr) ----------
            gath_stack = ExitStack()
            gathp = gath_stack.enter_context(
                tc.tile_pool(name="gathp", bufs=1))
            gath = gathp.tile([P, NP, WGP], f16, name="gath")
            gf_flat = gath[:].rearrange("p a b -> p (a b)")
            nc.gpsimd.memset(gf_flat[:, 0:NP * WGP], 0.0)

            # ---------- mask grid ----------
            oh_stack = ExitStack()
            ohp = oh_stack.enter_context(tc.tile_pool(name="ohgrid", bufs=1))
            oh = ohp.tile([P, G, WGP], bf16, name="oh")
            if WGP > COLS:
                flat = oh[:].rearrange("p a b -> p (a b)")
                tail = dataclasses.replace(
                    flat, offset=flat.offset + COLS,
                    ap=[flat.ap[0], [WGP, G], [1, WGP - COLS]])
                nc.gpsimd.memset(tail, 0.0)

            # ---------- anchors + prep ----------
            axh_stack = ExitStack()
            axp = axh_stack.enter_context(tc.tile_pool(name="axp", bufs=1))
            pA_stack = ExitStack()
            pA = pA_stack.enter_context(tc.tile_pool(name="pA", bufs=1))
            ax = [pA.tile([P, COLS], f32, name=f"ax{i}") for i in range(4)]
            PC0 = 384
            for u0, u1 in ((0, PC0), (PC0, COLS)):
                for i in range(4):
                    nc.sync.dma_start(ax[i][:, u0:u1],
                                      plane(anch, i)[:, u0:u1])
            axh = [axp.tile([P, COLS], f16, name=f"axh{i}") for i in range(4)]
            wah = pA.tile([P, COLS], f16, name="wah")
            hah = pA.tile([P, COLS], f16, name="hah")
            sa3 = pA.tile([P, COLS], f16, name="sa3")
            negq = persist.tile([P, COLS], f16, name="negq")
            for u0, u1 in ((0, PC0), (PC0, COLS)):
                sl = slice(u0, u1)
                for i in range(4):
                    nc.scalar.activation(axh[i][:, sl], ax[i][:, sl],
                                         AF.Identity, scale=0.5)
                nc.vector.tensor_tensor(out=wah[:, sl], in0=axh[2][:, sl],
                                        in1=axh[0][:, sl], op=AL.subtract)
                nc.vector.tensor_tensor(out=hah[:, sl], in0=axh[3][:, sl],
                                        in1=axh[1][:, sl], op=AL.subtract)
                nc.vector.tensor_tensor(out=sa3[:, sl], in0=wah[:, sl],
                                        in1=hah[:, sl], op=AL.mult)
                nc.scalar.activation(negq[:, sl], sa3[:, sl], AF.Identity,
                                     scale=-1.0 / 3.0)
            ca = persist.tile([P, COLS], f16, name="ca")
            ya = persist.tile([P, COLS], f16, name="ya")
            nc.vector.tensor_tensor(out=ca[:], in0=axh[0][:], in1=axh[2][:],
                                    op=AL.add)
            nc.vector.tensor_tensor(out=ya[:], in0=axh[1][:], in1=axh[3][:],
                                    op=AL.add)
            wae = pA.tile([P, COLS], f32, name="wae")
            hae = pA.tile([P, COLS], f32, name="hae")
            nc.vector.tensor_scalar(out=wae[:], in0=wah[:], scalar1=2.0,
                                    scalar2=EPS, op0=AL.mult, op1=AL.add)
            nc.vector.tensor_scalar(out=hae[:], in0=hah[:], scalar1=2.0,
                                    scalar2=EPS, op0=AL.mult, op1=AL.add)
            iwa = persist.tile([P, COLS], f16, name="iwa")
            iha = persist.tile([P, COLS], f16, name="iha")
            with nc.allow_low_precision(reason="f16 reg targets"):
                nc.vector.reciprocal(iwa[:], wae[:])
                nc.vector.reciprocal(iha[:], hae[:])
            La = persist.tile([P, COLS], f16, name="La")
            Ha = persist.tile([P, COLS], f16, name="Ha")
            nc.scalar.activation(La[:], wae[:], AF.Ln)
            nc.scalar.activation(Ha[:], hae[:], AF.Ln)
            pA_stack.close()

            # ---------- psum accumulators (phase A) ----------
            acc_stack = ExitStack()
            accp = acc_stack.enter_context(
                tc.tile_pool(name="accp", bufs=1, space="PSUM"))
            nsumP = accp.tile([1, 512], f32, name="nsumP")

            gp_stack = ExitStack()
            psum_t = gp_stack.enter_context(
                tc.tile_pool(name="psum_t", bufs=3, space="PSUM"))
            psum_g = gp_stack.enter_context(
                tc.tile_pool(name="psum_g", bufs=2, space="PSUM"))
            ohtp = gp_stack.enter_context(tc.tile_pool(name="ohtp", bufs=2))

            ohbase = oh[:].rearrange("p a b -> p (a b)")

            def emit_group(g, bs):
                nch = len(bs)
                c0g = g * GW
                pt = psum_t.tile([P, 128 * nch], bf16, name="pt")
                for q, b in enumerate(bs):
                    src = dataclasses.replace(
                        ohbase,
                        offset=ohbase.offset + (NB * b) * WGP + c0g,
                        ap=[ohbase.ap[0], [WGP, NB], [1, MROW]])
                    nc.tensor.transpose(
                        pt[:, 128 * q:128 * q + 128], src, identB[:])
                ohT = ohtp.tile([P, 128 * nch], bf16, name="ohT")
                nc.scalar.copy(ohT[:], pt[:])
                gp = psum_g.tile([P, MROW * NP], f32, name="gp")
                for q, b in enumerate(bs):
                    l = ohT[:, 128 * q:128 * q + 128]
                    nc.tensor.matmul(out=gp[:], lhsT=l, rhs=ttb_hl(b, 0),
                                     start=(q == 0), stop=False)
                    nc.tensor.matmul(out=gp[:], lhsT=l, rhs=ttb_hl(b, 1),
                                     start=False, stop=(q == nch - 1))
                dst = dataclasses.replace(
                    gf_flat, offset=gf_flat.offset + c0g,
                    ap=[gf_flat.ap[0], [1, MROW], [WGP, NP]])
                nc.scalar.copy(dst, gp[:])

            # ---------- phase A ----------
            fns_stack = ExitStack()
            fs1 = fns_stack.enter_context(tc.tile_pool(name="fns", bufs=1))
            fxc = [fs1.tile([P, COLS], f16, name=f"fxc{i}") for i in range(2)]
            fsg = fs1.tile([P, COLS], f16, name="fsg")
            fsp = fsg
            fs2 = fs1.tile([P, COLS], f16, name="fs2")
            fNo = [fs1.tile([P, COLS], f16, name=f"No{i}") for i in range(2)]

            def emit_focal_class(c):
                # No = -N(x) = (1-sgn)^2 * ln(sgn), sgn = sigmoid(-x)
                i2 = c % 2
                xc = fxc[i2]
                nc.sync.dma_start(xc[:], plane(clsp, c))
                nc.scalar.activation(fsg[:], xc[:], AF.Sigmoid, scale=-1.0)
                nc.scalar.activation(fs2[:], fsg[:], AF.Square,
                                     scale=-1.0, bias=1.0)
                nc.scalar.activation(fsp[:], fsg[:], AF.Ln)
                nc.gpsimd.tensor_tensor(out=fNo[i2][:], in0=fs2[:],
                                        in1=fsp[:], op=AL.mult)
                for k, (u0, u1) in enumerate(CH):
                    nc.tensor.matmul(
                        out=nsumP[:, 0:u1 - u0], lhsT=onesH[:],
                        rhs=fNo[i2][:, u0:u1],
                        start=(c == 0 and k == 0),
                        stop=(c == C - 1 and k == len(CH) - 1))

            with tc.tile_pool(name="jtmp", bufs=1) as jt:
                ltx2 = [jt.tile([P, MAXW], f16, name=f"ltx{i}") for i in range(2)]
                ux2 = [jt.tile([P, MAXW], f16, name=f"ux{i}") for i in range(2)]
                lty2 = [jt.tile([P, MAXW], f16, name=f"lty{i}") for i in range(2)]
                vy2 = [jt.tile([P, MAXW], f16, name=f"vy{i}") for i in range(2)]
                wxp2 = ltx2
                wyn2 = lty2
                int3 = [jt.tile([P, MAXW], f16, name=f"int{i}") for i in range(3)]
                m3 = [jt.tile([P, MAXW], f16, name=f"m{i}") for i in range(3)]
                pend = None
                nsub = [0]

                def flush_tail():
                    nonlocal pend
                    if pend is None:
                        return
                    j, c0, c1, mj, k3 = pend
                    pend = None
                    w = c1 - c0
                    eb = eb3[k3 % 2]
                    nc.vector.tensor_scalar(out=oh[:, j, c0:c1],
                                            in0=mj[:, 0:w],
                                            scalar1=gf(RF_NAB3, j),
                                            scalar2=None, op0=AL.is_le)
                    nc.vector.tensor_scalar(out=eb[:, 0:w], in0=mj[:, 0:w],
                                            scalar1=gf(RF_NAB3, j),
                                            scalar2=30000.0,
                                            op0=AL.is_le, op1=AL.mult)
                    nc.vector.tensor_tensor(out=negq[:, c0:c1],
                                            in0=negq[:, c0:c1],
                                            in1=eb[:, 0:w], op=AL.subtract)

                def emit_sub(j, c0, c1):
                    sl = slice(c0, c1)
                    w = c1 - c0
                    k2 = nsub[0] % 2
                    k3 = nsub[0] % 3
                    ltx = ltx2[k2]; ux = ux2[k2]
                    lty = lty2[k2]; vy = vy2[k2]
                    wxp = wxp2[k2]; wyn = wyn2[k2]
                    itg = int3[k3]; mj = m3[k3]
                    v = nc.vector
                    v.tensor_scalar(out=ltx[:, 0:w], in0=axh[0][:, sl],
                                    scalar1=gf(RF_GX1, j), scalar2=gf(RF_GX2, j),
                                    op0=AL.max, op1=AL.min)
                    v.tensor_scalar(out=ux[:, 0:w], in0=axh[2][:, sl],
                                    scalar1=gf(RF_GX2, j), scalar2=gf(RF_GX1, j),
                                    op0=AL.min, op1=AL.max)
                    v.tensor_scalar(out=lty[:, 0:w], in0=axh[1][:, sl],
                                    scalar1=gf(RF_GY1, j), scalar2=gf(RF_GY2, j),
                                    op0=AL.max, op1=AL.min)
                    v.tensor_scalar(out=vy[:, 0:w], in0=axh[3][:, sl],
                                    scalar1=gf(RF_GY2, j), scalar2=gf(RF_GY1, j),
                                    op0=AL.min, op1=AL.max)
                    v.tensor_tensor(out=wxp[:, 0:w], in0=ux[:, 0:w],
                                    in1=ltx[:, 0:w], op=AL.subtract)
                    v.tensor_tensor(out=wyn[:, 0:w], in0=lty[:, 0:w],
                                    in1=vy[:, 0:w], op=AL.subtract)
                    v.tensor_tensor(out=itg[:, 0:w], in0=wxp[:, 0:w],
                                    in1=wyn[:, 0:w], op=AL.mult)
                    flush_tail()
                    nc.gpsimd.tensor_tensor(out=mj[:, 0:w], in0=itg[:, 0:w],
                                            in1=negq[:, sl], op=AL.subtract)
                    return mj, k3

                for j in range(G):
                    c0, c1 = win[j]
                    s0 = c0
                    while s0 < c1:
                        s1 = min(s0 + MAXW, c1)
                        mj, k3 = emit_sub(j, s0, s1)
                        pend = (j, s0, s1, mj, k3)
                        nsub[0] += 1
                        s0 = s1
                    if j % NB == NB - 1:
                        emit_focal_class(j // NB)
                    if j in ready:
                        flush_tail()
                        for g, bs in ready[j]:
                            emit_group(g, bs)
                flush_tail()
            fns_stack.close()

            # read out nsum; close gather pools
            nsum1 = persist.tile([1, 1], f32, name="nsum1")
            nscr = persist.tile([1, 512], f32, name="nscr")
            nc.scalar.activation(nscr[:], nsumP[:], AF.Identity,
                                 accum_out=nsum1[:])
            gp_stack.close()
            acc_stack.close()
            axh_stack.close()
            oh_stack.close()

            def gpl(idx):
                return gath[:, idx, 0:COLS]

            pos = gpl(4)
            labf = gpl(5)

            nposA = persist.tile([P, 1], f32, name="nposA")
            with tc.tile_pool(name="pscrp", bufs=1) as pscrp:
                pscr = pscrp.tile([P, COLS], f16, name="pscr")
                nc.scalar.activation(pscr[:], pos, AF.Identity,
                                     accum_out=nposA[:])

            if debug:
                dview = dbg[:].rearrange("p (n w) -> p n w", n=4)
                nc.scalar.copy(dview[:, 0, :], pos)
                nc.scalar.copy(dview[:, 1, :], labf)
                nc.scalar.copy(dview[:, 2, :], gpl(0))
                nc.scalar.copy(dview[:, 3, :], gpl(2))

            acc2_stack = ExitStack()
            accp2 = acc2_stack.enter_context(
                tc.tile_pool(name="accp2", bufs=1, space="PSUM"))
            sl1P = accp2.tile([1, 512], f32, name="sl1P")
            roP = accp2.tile([1, 512], f32, name="roP")

            # ---------- reg smooth-L1 ----------
            with ExitStack() as rctx:
                rp_p = rctx.enter_context(tc.tile_pool(name="rp", bufs=2))
                rs = rctx.enter_context(tc.tile_pool(name="rs", bufs=1))
                dv = [rs.tile([P, COLS], f16, name=f"dv{i}") for i in range(2)]
                rt = [rs.tile([P, COLS], f16, name=f"rt{i}") for i in range(2)]
                ef = [rs.tile([P, COLS], f16, name=f"ef{i}") for i in range(2)]
                qf = [rs.tile([P, COLS], f16, name=f"qf{i}") for i in range(2)]
                qm = [rs.tile([P, COLS], f16, name=f"qm{i}") for i in range(2)]
                cm = [rs.tile([P, COLS], f16, name=f"cm{i}") for i in range(2)]
                q2 = [rs.tile([P, COLS], f16, name=f"q2{i}") for i in range(2)]
                t2 = [rs.tile([P, COLS], f16, name=f"t2{i}") for i in range(2)]
                so = [rs.tile([P, COLS], f16, name=f"so{i}") for i in range(2)]
                for k, (gi, ctr, inv, lg) in enumerate((
                        (0, ca, iwa, None), (1, ya, iha, None),
                        (2, None, None, La), (3, None, None, Ha))):
                    i2 = k % 2
                    g = gpl(gi)
                    if lg is None:
                        nc.vector.tensor_tensor(out=dv[i2][:], in0=g,
                                                in1=ctr[:], op=AL.subtract)
                        nc.vector.tensor_tensor(out=rt[i2][:], in0=dv[i2][:],
                                                in1=inv[:], op=AL.mult)
                    else:
                        nc.vector.tensor_tensor(out=rt[i2][:], in0=g,
                                                in1=lg[:], op=AL.subtract)
                    rp = rp_p.tile([P, COLS], f16, name="rp")
                    nc.sync.dma_start(rp[:], plane(regp, k))
                    nc.vector.tensor_tensor(out=ef[i2][:], in0=rp[:],
                                            in1=rt[i2][:], op=AL.subtract)
                    nc.scalar.activation(qf[i2][:], ef[i2][:], AF.Abs)
                    nc.gpsimd.tensor_tensor(out=qm[i2][:], in0=qf[i2][:],
                                            in1=pos, op=AL.mult)
                    nc.vector.tensor_scalar(out=cm[i2][:], in0=qm[i2][:],
                                            scalar1=BETA, scalar2=None,
                                            op0=AL.min)
                    nc.vector.tensor_scalar(out=q2[i2][:], in0=qm[i2][:],
                                            scalar1=2.0, scalar2=None,
                                            op0=AL.mult)
                    nc.vector.tensor_tensor(out=t2[i2][:], in0=q2[i2][:],
                                            in1=cm[i2][:], op=AL.subtract)
                    nc.vector.tensor_tensor(out=so[i2][:], in0=cm[i2][:],
                                            in1=t2[i2][:], op=AL.mult)
                    for kk, (u0, u1) in enumerate(CH):
                        nc.tensor.matmul(
                            out=sl1P[:, 0:u1 - u0], lhsT=onesH[:],
                            rhs=so[i2][:, u0:u1],
                            start=(k == 0 and kk == 0),
                            stop=(k == 3 and kk == len(CH) - 1))

            # ---------- focal corr ----------
            with ExitStack() as fctx:
                fs = fctx.enter_context(tc.tile_pool(name="fsc", bufs=1))
                x_lab = fs.tile([P, COLS], f16, name="x_lab")
                nc.scalar.activation(x_lab[:], x_lab[:], AF.MemsetZero,
                                     zero_input=True) if False else \
                    nc.gpsimd.memset(x_lab[:], 0.0)
                mk = [fs.tile([P, COLS], i16, name=f"mk{i}") for i in range(2)]
                cxc = [fs.tile([P, COLS], f16, name=f"cxc{i}") for i in range(2)]
                for c in range(C):
                    i2 = c % 2
                    nc.sync.dma_start(cxc[i2][:], plane(clsp, c))
                    nc.vector.tensor_scalar(out=mk[i2][:], in0=labf,
                                            scalar1=float(c), scalar2=None,
                                            op0=AL.is_equal)
                    nc.vector.copy_predicated(out=x_lab[:], mask=mk[i2][:],
                                              data=cxc[i2][:])
                # R' = -R = (-P)/3 - (-N); -P = sgn^2*ln(sg); -N = sg^2*ln(sgn)
                sgn = fs.tile([P, COLS], f16, name="sgn")
                sgl = fs.tile([P, COLS], f16, name="sgl")
                lgn = fs.tile([P, COLS], f16, name="lgn")
                lgl = fs.tile([P, COLS], f16, name="lgl")
                a2 = fs.tile([P, COLS], f16, name="a2")
                Pl = fs.tile([P, COLS], f16, name="Pl")
                b2 = fs.tile([P, COLS], f16, name="b2")
                Nl = fs.tile([P, COLS], f16, name="Nl")
                Rl = fs.tile([P, COLS], f16, name="Rl")
                Ro = fs.tile([P, COLS], f16, name="Ro")
                nc.scalar.activation(sgn[:], x_lab[:], AF.Sigmoid, scale=-1.0)
                nc.scalar.activation(sgl[:], x_lab[:], AF.Sigmoid)
                nc.scalar.activation(lgn[:], sgn[:], AF.Ln)
                nc.scalar.activation(lgl[:], sgl[:], AF.Ln)
                nc.vector.tensor_tensor(out=a2[:], in0=sgn[:], in1=sgn[:],
                                        op=AL.mult)
                nc.vector.tensor_tensor(out=Pl[:], in0=a2[:], in1=lgl[:],
                                        op=AL.mult)
                nc.vector.tensor_tensor(out=b2[:], in0=sgl[:], in1=sgl[:],
                                        op=AL.mult)
                nc.vector.tensor_tensor(out=Nl[:], in0=b2[:], in1=lgn[:],
                                        op=AL.mult)
                nc.vector.scalar_tensor_tensor(out=Rl[:], in0=Pl[:],
                                               scalar=1.0 / 3.0, in1=Nl[:],
                                               op0=AL.mult, op1=AL.subtract)
                nc.vector.tensor_tensor(out=Ro[:], in0=Rl[:], in1=pos,
                                        op=AL.mult)
                for kk, (u0, u1) in enumerate(CH):
                    nc.tensor.matmul(
                        out=roP[:, 0:u1 - u0], lhsT=onesH[:],
                        rhs=Ro[:, u0:u1],
                        start=(kk == 0), stop=(kk == len(CH) - 1))

            # ---------- final reduce ----------
            sl11 = persist.tile([1, 1], f32, name="sl11")
            ro1 = persist.tile([1, 1], f32, name="ro1")
            scr1 = persist.tile([1, 512], f32, name="scr1")
            nc.scalar.activation(scr1[:], sl1P[:], AF.Identity,
                                 accum_out=sl11[:])
            scr2 = persist.tile([1, 512], f32, name="scr2")
            nc.scalar.activation(scr2[:], roP[:], AF.Identity,
                                 accum_out=ro1[:])
            acc2_stack.close()
            gath_stack.close()
            with tc.tile_pool(name="psum_f", bufs=1, space="PSUM") as pf:
                fps = pf.tile([1, 1], f32, name="fps")
                nc.tensor.matmul(out=fps[:], lhsT=ones[:], rhs=nposA[:],
                                 start=True, stop=True)
                osb = persist.tile([1, 4], f32, name="osb")
                nc.scalar.copy(osb[:, 0:1], fps[:])
                nc.scalar.copy(osb[:, 1:2], sl11[:])
                nc.scalar.copy(osb[:, 2:3], nsum1[:])
                nc.scalar.copy(osb[:, 3:4], ro1[:])
                nc.sync.dma_start(out[:], osb[:])

    return nc


# ---------------- host side ----------------

def prep_host(cls_preds, reg_preds, anchors, gt_boxes, gt_labels):
    B, A, _ = cls_preds.shape
    COLS = A // P
    NG = (COLS + GW - 1) // GW
    f = np.float32
    a = anchors.astype(np.float64)
    SA = (a[:, 2] - a[:, 0]) * (a[:, 3] - a[:, 1])
    order = np.argsort(SA, kind="stable")
    SA_s = SA[order]

    def cm_layout(x):
        return np.ascontiguousarray(x.reshape(COLS, P).T)

    a_s = a[order]
    anch = np.stack([cm_layout(a_s[:, i].astype(f)) for i in range(4)], 0)
    anch = anch.reshape(4, A)

    import ml_dtypes
    bfc = lambda v: f(f(v).astype(ml_dtypes.bfloat16))

    lo_cols = np.zeros((B, G), np.int64)
    hi_cols = np.zeros((B, G), np.int64)
    maps = []
    for b in range(B):
        gb = gt_boxes[b].astype(np.float64)
        lab = gt_labels[b].astype(np.int64)
        SG = (gb[:, 2] - gb[:, 0]) * (gb[:, 3] - gb[:, 1])
        gorder = np.argsort(SG, kind="stable")
        gb, lab, SG = gb[gorder], lab[gorder], SG[gorder]
        lo = np.searchsorted(SA_s, SG / 2.0)
        hi = np.searchsorted(SA_s, 2.0 * SG, side="right")
        lo_cols[b] = lo // P
        hi_cols[b] = np.minimum((hi + P - 1) // P, COLS)

        gx1, gy1, gx2, gy2 = gb[:, 0], gb[:, 1], gb[:, 2], gb[:, 3]
        rows = np.zeros((NF, G), f)
        rows[RF_GX1] = f(gx1 * 0.5)
        rows[RF_GY1] = f(gy1 * 0.5)
        rows[RF_GX2] = f(gx2 * 0.5)
        rows[RF_GY2] = f(gy2 * 0.5)
        rows[RF_NAB3] = f(-(SG / 12.0))
        xg = (gx1 + gx2) / 2.0
        yg = (gy1 + gy2) / 2.0
        lwg = np.log(gx2 - gx1)
        lhg = np.log(gy2 - gy1)
        pay = np.zeros((2, G, NP), f)
        for r, v in ((0, xg), (1, yg), (2, lwg), (3, lhg)):
            h = bfc(v)
            pay[0, :, r] = h
            pay[1, :, r] = bfc(v - h)
        pay[0, :, 4] = 1.0
        pay[0, :, 5] = lab.astype(f)
        pay /= 32768.0
        TBW = MROW * NP
        tt = np.zeros((P, 2 * NBLK * TBW), f)
        for bb_ in range(NBLK):
            for hl in (0, 1):
                t = np.zeros((P, TBW), f)
                for r in range(NB):
                    j = NB * bb_ + r
                    for i in range(MROW):
                        t[MROW * r + i, NP * i:NP * i + NP] = pay[hl, j]
                tt[:, (2 * bb_ + hl) * TBW:(2 * bb_ + hl + 1) * TBW] = t

        clspb = cls_preds[b][order].astype(np.float16)
        regpb = reg_preds[b][order].astype(np.float16)
        clsp = np.stack([cm_layout(clspb[:, i]) for i in range(C)], 0)
        regp = np.stack([cm_layout(regpb[:, i]) for i in range(4)], 0)
        maps.append({"anch": anch, "clsp": clsp.reshape(C, A),
                     "regp": regp.reshape(4, A),
                     "gtf": rows.reshape(1, -1), "ttbs": tt})

    win = []
    spans = []
    for bb_ in range(NBLK):
        js = range(NB * bb_, NB * bb_ + NB)
        c0 = int(min(lo_cols[:, j].min() for j in js))
        c1 = int(max(hi_cols[:, j].max() for j in js))
        c0 = (max(0, c0) // GW) * GW
        c1 = min(COLS, ((max(c1, c0) + GW - 1) // GW) * GW)
        spans.append((c0, c1))
    for j in range(G):
        win.append(spans[j // NB])
    groups = []
    for g in range(NG):
        g0, g1 = g * GW, min((g + 1) * GW, COLS)
        bs = [bb_ for bb_, (c0, c1) in enumerate(spans)
              if c0 < g1 and c1 > g0]
        groups.append((g, bs))
    return maps, win, groups


def finish(partials):
    f = np.float32
    npos = f(0); sl1 = f(0); nsum = f(0); corr = f(0)
    for p in partials:
        p = p.reshape(4)
        npos += f(p[0]); sl1 += f(p[1]); nsum -= f(p[2]); corr -= f(p[3])
    denom = max(float(npos), 1.0)
    if npos > 0:
        cls_loss = f(0.75) * (nsum + corr) / f(denom)
        reg_loss = sl1 / f(2 * BETA) / f(denom)
    else:
        cls_loss = f(0.0); reg_loss = f(0.0)
    return np.float32(cls_loss), np.float32(reg_loss)


# ---------------- self-contained kernel entry ----------------

_CACHE = {}


def _get_fn(nc, n_cores=8):
    import jax
    from jax.sharding import Mesh, PartitionSpec, NamedSharding
    from jax.experimental.shard_map import shard_map
    from concourse.bass2jax import (_bass_exec_p, install_neuronx_cc_hook,
                                    partition_id_tensor)
    install_neuronx_cc_hook()
    in_names, out_names, out_avals, zero_shapes = [], [], [], []
    partition_name = (nc.partition_id_tensor.name
                      if nc.partition_id_tensor else None)
    for alloc in nc.m.functions[0].allocations:
        if not isinstance(alloc, mybir.MemoryLocationSet):
            continue
        name = alloc.memorylocations[0].name
        if alloc.kind == "ExternalInput":
            if name != partition_name:
                in_names.append(name)
        elif alloc.kind == "ExternalOutput":
            out_names.append(name)
            shape = tuple(alloc.tensor_shape)
            dtype = mybir.dt.np(alloc.dtype)
            out_avals.append(jax.core.ShapedArray(shape, dtype))
            zero_shapes.append((shape, dtype))
    n_params = len(in_names)
    n_outs = len(out_avals)
    all_in_names = in_names + out_names + ([partition_name]
                                           if partition_name else [])
    donate = tuple(range(n_params, n_params + n_outs))

    def _body(*args):
        operands = list(args)
        if partition_name is not None:
            operands.append(partition_id_tensor())
        outs = _bass_exec_p.bind(
            *operands, out_avals=tuple(out_avals),
            in_names=tuple(all_in_names), out_names=tuple(out_names),
            lowering_input_output_aliases=(),
            sim_require_finite=True, sim_require_nnan=True, nc=nc)
        return tuple(outs)

    devices = jax.devices()[:n_cores]
    mesh = Mesh(np.asarray(devices), ("core",))
    in_specs = (PartitionSpec("core"),) * (n_params + n_outs)
    out_specs = (PartitionSpec("core"),) * len(out_names)
    fn = jax.jit(shard_map(_body, mesh=mesh, in_specs=in_specs,
                           out_specs=out_specs, check_rep=False),
                 donate_argnums=donate, keep_unused=True)
    sh = NamedSharding(mesh, PartitionSpec("core"))
    return (fn, in_names, out_names, out_avals, zero_shapes, sh, n_cores)


def kernel(cls_preds, reg_preds, anchors, gt_boxes, gt_labels):
    import jax
    cls_preds = np.asarray(cls_preds)
    reg_preds = np.asarray(reg_preds)
    anchors = np.asarray(anchors)
    gt_boxes = np.asarray(gt_boxes)
    gt_labels = np.asarray(gt_labels)
    B, A, _ = cls_preds.shape
    assert (B, A) == (8, 160000), (B, A)
    maps, win, groups = prep_host(cls_preds, reg_preds, anchors, gt_boxes,
                                  gt_labels)
    key = ("fn", tuple(win), tuple((g, tuple(bs)) for g, bs in groups))
    if key not in _CACHE:
        patch_tile_drain(1)
        nc = build(A, win, groups)
        split_sync_waits(nc)
        _CACHE.clear()
        _CACHE[key] = _get_fn(nc)
    fn, in_names, out_names, out_avals, zero_shapes, sh, n_cores = _CACHE[key]
    concat_in = [jax.device_put(
        np.concatenate([np.asarray(maps[c][nm]) for c in range(n_cores)],
                       axis=0), sh) for nm in in_names]
    zeros = [jax.device_put(
        np.zeros((n_cores * s[0], *s[1:]), d), sh) for s, d in zero_shapes]
    out_arrs = fn(*concat_in, *zeros)
    res = np.asarray(out_arrs[out_names.index("out")]).reshape(n_cores, 1, 4)
    partials = [res[c] for c in range(n_cores)]
    cls_loss, reg_loss = finish(partials)
    return cls_loss, reg_loss
